# revision 2
# baseline (speedup 1.0000x reference)
"""Trainium2 Bass kernel for nn_Consistent_loss_up_2 (scatter_memory).

Architecture (vs v1 baseline at 74.1us):
  * per-pair scatter calls (~100ns call overhead measured on HW; fine
    granularity lets loss chunks start early and keeps the tail short);
    pair 0 split into two single-batch half calls for an early start
  * mask folded into ONE DVE op: ix = (u >= 0.0235) * g; masked lanes
    produce ps = 0 -> idx = cofs < 0 -> scatter skips them
  * per-block index offsets + the -1045 rebase fused into a single DVE
    TT: stc_i16 = ps_f16 + cofsT_f16, cofsT a memset-built const
    (block b of 128 cols = 50*b - 1045); replaces the baseline's
    cofs-subtract DVE op and its ACT st pass
  * loss: e = refs - tbl on DVE (f16), then ACT Abs + two ACT
    accumulation passes (relu(12-a), sign(a-11.996)); ACT runs at full
    speed under concurrent local_scatter while DVE runs ~7x slow
  * value/ref offset = 13 (not 1024): refs = 60*ref + 13 sit at f16
    ulp 1/128..1/16, so the d<0.2 inclusion band is cut by <=1/32
    instead of 0.5 (a 1024 offset loses ~8% of the loss mass: the
    d~0.2 slots are the largest contributors); empty bins still
    auto-fail (a = refs >= 13 > 12); sign threshold 12-1/256 is an
    f32 bias that never ties the 1/128-grid |e| values
  * refs/vee/tbl in fp16 (halves ref DMA + loss cost)
  * gpsimd ucode library preloaded at t0; gpsimd queue carries only
    the 9 scatter calls

Sharding: data-parallel over batch B=128 across 8 cores (16 each);
each core returns [128,10] partial sums, host reduces.
"""

import numpy as np

from concourse import bacc, library_config, mybir, tile
from concourse.bass_utils import run_bass_kernel_spmd

B, H, W = 128, 256, 256
NCORES = 8
KPC = B // NCORES        # batches per core = 16
NPAIR = KPC // 2         # 8
KTBL = 200               # table cols per batch
TBL_W = KPC * KTBL       # 3200
OFF = 13.0
# loss chunks in batch units
CHUNKS = [(0, 4), (4, 8), (8, 12), (12, 14), (14, 15), (15, 16)]
NCHUNK = len(CHUNKS)

_cache = {}


def _build_bass():
    nc = bacc.Bacc("TRN2", target_bir_lowering=False)
    f32, i16, f16 = mybir.dt.float32, mybir.dt.int16, mybir.dt.float16
    Alu = mybir.AluOpType
    Act = mybir.ActivationFunctionType

    up_in = nc.dram_tensor("up_in", [KPC * H, W], f32, kind="ExternalInput")
    bcol_in = nc.dram_tensor("bcol_in", [128, 4], f32, kind="ExternalInput")
    refs_in = nc.dram_tensor("refs_in", [128, TBL_W], f16, kind="ExternalInput")
    vee_in = nc.dram_tensor("vee_in", [128, 1024], f16, kind="ExternalInput")
    ident_in = nc.dram_tensor("ident_in", [128, 128], f16, kind="ExternalInput")
    antid_in = nc.dram_tensor("antid_in", [128, 128], f16, kind="ExternalInput")
    negid_in = nc.dram_tensor("negid_in", [128, 128], f16, kind="ExternalInput")
    out = nc.dram_tensor("out", [128, 2 * NCHUNK], f32, kind="ExternalOutput")

    with tile.TileContext(nc) as tc:
        with (
            tc.tile_pool(name="const", bufs=1) as constp,
            tc.tile_pool(name="big", bufs=1) as bigp,
            tc.tile_pool(name="ut", bufs=6) as utp,
            tc.tile_pool(name="g", bufs=6) as gp,
            tc.tile_pool(name="ix", bufs=3) as ixp,
            tc.tile_pool(name="psum", bufs=4, space="PSUM") as psp,
            tc.tile_pool(name="lpsum", bufs=2, space="PSUM") as lpsp,
            tc.tile_pool(name="loss", bufs=5) as lossp,
        ):
            # gpsimd: preload the scatter ucode library; nothing else
            # runs on this queue except the scatter calls
            nc.gpsimd.load_library(library_config.local_scatter)

            # cofsT const: block b (128 cols) = 50*b - 1045 (idx rebase:
            # unmasked ps = bin + 934 in [1045..1094] -> idx in [0,400))
            cofsT = constp.tile([128, 1024], f16, name="cofsT")
            for b in range(8):
                nc.vector.memset(cofsT[:, b * 128:(b + 1) * 128],
                                 50.0 * b - 1045.0)

            ident = constp.tile([128, 128], f16, name="ident")
            nc.scalar.dma_start(ident[:], ident_in[:])
            antid = constp.tile([128, 128], f16, name="antid")
            nc.scalar.dma_start(antid[:], antid_in[:])
            bcol = constp.tile([128, 4], f32, name="bcol")
            nc.scalar.dma_start(bcol[:], bcol_in[:])
            vee = constp.tile([128, 1024], f16, name="vee")
            negid = constp.tile([128, 128], f16, name="negid")
            refs = bigp.tile([128, TBL_W], f16, name="refs")

            stc = bigp.tile([128, 8192], i16, name="stc")
            tbl = bigp.tile([128, TBL_W], f16, name="tbl")
            parts = bigp.tile([128, 2 * NCHUNK], f32, name="parts")

            uts = {}

            def fetch_ut(p):
                uts[p] = utp.tile([128, 4 * W], f32, tag="ut", name=f"ut{p}")
                src = up_in[p * 2 * H:(p + 1) * 2 * H, :].rearrange(
                    "(q p) w -> p q w", q=4
                )
                dst = uts[p][:].rearrange("p (q w) -> p q w", q=4)
                nc.sync.dma_start(dst, src)

            def fetch_ut0_half(half):
                # pair 0 is loaded as two single-batch halves so the first
                # scatter's chain starts earlier
                src = up_in[half * H:(half + 1) * H, :].rearrange(
                    "(q p) w -> p q w", q=2
                )
                dst = uts[0][:, half * 512:(half + 1) * 512].rearrange(
                    "p (q w) -> p q w", q=2
                )
                nc.sync.dma_start(dst, src)

            def emit_g(p, half=None):
                sl = slice(0, 1024) if half is None else \
                    slice(half * 512, half * 512 + 512)
                t = gp.tile([128, sl.stop - sl.start], f16, tag="g",
                            name=f"g{p}_{sl.start}")
                nc.scalar.activation(t[:], uts[p][:, sl], Act.Copy,
                                     bias=1044.0, scale=50.0)
                return t

            def emit_ix(p, g, half=None):
                # pre-scatter pairs: one fused DVE op (f32 compare+mult)
                sl = slice(0, 1024) if half is None else \
                    slice(half * 512, half * 512 + 512)
                t = ixp.tile([128, sl.stop - sl.start], f16, tag="ix",
                             name=f"ix{p}_{sl.start}")
                nc.vector.scalar_tensor_tensor(
                    out=t[:], in0=uts[p][:, sl], scalar=0.0235, in1=g[:],
                    op0=Alu.is_ge, op1=Alu.mult,
                )
                return t

            def emit_sgn(p):
                # mask as ACT Sign(u - 0.0235) = +-1 (scatter-immune)
                t = gp.tile([128, 1024], f16, tag="g", name=f"sgn{p}")
                nc.scalar.activation(t[:], uts[p][:], Act.Sign,
                                     bias=bcol[:, 3:4], scale=1.0)
                return t

            def emit_ix_sgn(p, g, sgn):
                # masked lanes: -g -> idx stays negative after cofs
                t = ixp.tile([128, 1024], f16, tag="ix", name=f"ixs{p}")
                nc.vector.tensor_tensor(
                    out=t[:], in0=g[:], in1=sgn[:], op=Alu.mult,
                )
                return t

            def emit_transposes(p, ix, half=None):
                n = 512 if half is not None else 1024
                ps = psp.tile([128, n], f16, tag="ps",
                              name=f"ps{p}_{half}", space="PSUM")
                for kk in range(n // 512):
                    o = kk * 512
                    for dst_o, src_o, w in (
                        (o + 0, o + 0, antid),      # L_jt0
                        (o + 256, o + 128, antid),  # L_jt1
                        (o + 128, o + 256, ident),  # R_jt0
                        (o + 384, o + 384, ident),  # R_jt1
                    ):
                        nc.tensor.transpose(
                            ps[:, dst_o:dst_o + 128],
                            ix[:, src_o:src_o + 128], w[:],
                        )
                return ps

            def emit_stc(p, ps, half=None):
                base = p * 1024 + (0 if half is None else half * 512)
                n = 512 if half is not None else 1024
                src_ap = ps[:] if ps.shape[1] == n else \
                    ps[:, half * 512:half * 512 + 512]
                nc.vector.tensor_tensor(
                    out=stc[:, base:base + n], in0=src_ap,
                    in1=cofsT[:, 0:n], op=Alu.add,
                )

            def emit_call(p, half=None):
                # one scatter call per pair (or per batch for halves)
                if half is None:
                    i0, nidx, nel = p * 1024, 1024, 2 * KTBL
                    t0 = p * 2 * KTBL
                else:
                    i0, nidx, nel = p * 1024 + half * 512, 512, KTBL
                    t0 = p * 2 * KTBL + half * KTBL
                nc.gpsimd.local_scatter(
                    tbl[:, t0:t0 + nel],
                    vee[:, 0:nidx],
                    stc[:, i0:i0 + nidx],
                    channels=128,
                    num_elems=nel,
                    num_idxs=nidx,
                )

            def emit_loss(c, tensor_e=False):
                k0, k1 = CHUNKS[c]
                a0, a1 = k0 * KTBL, k1 * KTBL
                n = a1 - a0
                if tensor_e:
                    # e = refs - tbl built on the Tensor engine (immune to
                    # scatter contention): identity-copy + neg-identity acc.
                    # Matmul outputs must stay within one PSUM bank (512
                    # f32), so emit per-512-col slices.
                    e = lpsp.tile([128, n], f32, tag="le", name=f"le{c}",
                                  space="PSUM")
                    for s0 in range(0, n, 512):
                        s1 = min(s0 + 512, n)
                        nc.tensor.matmul(e[:, s0:s1], ident[:],
                                         refs[:, a0 + s0:a0 + s1],
                                         start=True, stop=False,
                                         skip_group_check=True)
                        nc.tensor.matmul(e[:, s0:s1], negid[:],
                                         tbl[:, a0 + s0:a0 + s1],
                                         start=False, stop=True,
                                         skip_group_check=True)
                else:
                    e = lossp.tile([128, n], f16, tag="e", name=f"e{c}")
                    nc.vector.tensor_tensor(
                        out=e[:], in0=refs[:, a0:a1], in1=tbl[:, a0:a1],
                        op=Alu.subtract,
                    )
                a = lossp.tile([128, n], f16, tag="a", name=f"a{c}")
                nc.scalar.activation(a[:], e[:], Act.Abs, bias=bcol[:, 2:3])
                r1 = lossp.tile([128, n], f32, tag="r1", name=f"r1_{c}")
                nc.scalar.activation(
                    r1[:], a[:], Act.Relu, bias=bcol[:, 0:1], scale=-1.0,
                    accum_out=parts[:, 2 * c:2 * c + 1],
                )
                # count pass on DVE, parallel with the ACT relu pass:
                # N = sum(a < 11.996) directly replaces the sign trick
                nb = lossp.tile([128, n], f16, tag="nb", name=f"nb{c}")
                nc.vector.tensor_scalar(
                    nb[:], a[:], 11.99609375, None, op0=Alu.is_lt,
                )
                nc.vector.tensor_reduce(
                    out=parts[:, 2 * c + 1:2 * c + 2], in_=nb[:],
                    axis=mybir.AxisListType.X, op=Alu.add,
                )
                if c == NCHUNK - 1:
                    nc.scalar.dma_start(out[:], parts[:])

            # ---------------- pipeline ----------------
            # DMA engines round-robin across all in-flight transfers, so
            # the first batch completes fastest when little else is
            # queued: only up0a/up0b start at t0; later fetches are gated
            # behind tiny dmas that depend on earlier batches landing
            thr = constp.tile([128, 8], f32, name="thr")
            uts[0] = utp.tile([128, 4 * W], f32, tag="ut", name="ut0")
            fetch_ut0_half(0)
            fetch_ut0_half(1)
            nc.sync.dma_start(thr[:, 0:2], uts[0][:, 0:2])
            fetch_ut(1)

            # pair 0 half a (batch k0) -> first scatter call
            g0a = emit_g(0, half=0)
            nc.scalar.dma_start(vee[:], vee_in[:])
            nc.scalar.dma_start(negid[:], negid_in[:])
            ix0a = emit_ix(0, g0a, half=0)
            ps0a = emit_transposes(0, ix0a, half=0)
            g0b = emit_g(0, half=1)
            emit_stc(0, ps0a, half=0)
            emit_call(0, half=0)
            nc.sync.dma_start(thr[:, 2:4], uts[1][:, 0:2])
            fetch_ut(2)

            # pair 0 half b (batch k1)
            ix0b = emit_ix(0, g0b, half=1)
            ps0b = emit_transposes(0, ix0b, half=1)
            g1 = emit_g(1)
            sgn1 = emit_sgn(1)
            emit_stc(0, ps0b, half=1)
            emit_call(0, half=1)
            nc.sync.dma_start(thr[:, 4:6], uts[2][:, 0:2])
            fetch_ut(3)

            # pairs 1..7; pairs >= 2 use the ACT Sign mask (sgn, emitted
            # two pairs ahead) so the per-pair DVE cost under concurrent
            # scatter is one f16 TT; loss chunks are emitted where their
            # ACT passes cannot delay a later pair's g/sgn
            gs = {1: (g1, sgn1),
                  2: (emit_g(2), emit_sgn(2))}
            for p in range(1, NPAIR):
                g_t, sgn_t = gs.pop(p)
                if sgn_t is None:
                    ix_t = emit_ix(p, g_t)
                else:
                    ix_t = emit_ix_sgn(p, g_t, sgn_t)
                ps_t = emit_transposes(p, ix_t)
                if p == 1:
                    nc.scalar.dma_start(refs[:, 0:1600], refs_in[:, 0:1600])
                if p == 2:
                    nc.scalar.dma_start(refs[:, 1600:], refs_in[:, 1600:])
                if p + 3 <= NPAIR - 1:
                    fetch_ut(p + 3)
                if p + 2 <= NPAIR - 1:
                    gs[p + 2] = (emit_g(p + 2), emit_sgn(p + 2))
                if p == NPAIR - 1:
                    emit_stc(p, ps_t, half=0)
                    emit_call(p, half=0)
                    emit_stc(p, ps_t, half=1)
                    emit_call(p, half=1)
                else:
                    emit_stc(p, ps_t)
                    emit_call(p)
            # all loss chunks emitted after the pipeline, pushed behind
            # the pipeline in the scheduler's clock via tile_wait_until
            # (runtime semaphores still start each chunk as soon as its
            # scatter call completes, so chunks 0-2 overlap calls 5-7)
            for c in range(NCHUNK):
                with tc.tile_wait_until(0.05 + 0.004 * c):
                    emit_loss(c, tensor_e=True)

    nc.compile()
    return nc


def _host_constants():
    n = np.arange(256)
    blk = np.where(n < 128, n + 1, n - 128).astype(np.float64) + OFF
    blk[128] = 25000.0
    vee = np.tile(blk, 4).astype(np.float16)
    vee = np.ascontiguousarray(np.broadcast_to(vee, (128, 1024)))
    ident = np.eye(128, dtype=np.float16)
    antid = np.ascontiguousarray(ident[::-1, :])
    return vee, ident, antid


def _prep_refs(left, right):
    """[128, 3200] f16 per core; table col = k*200 + blk*50 + bin' with
    blk in [L_jt0, R_jt0, L_jt1, R_jt1]; channel = j mod 128; values
    60*ref + OFF."""
    lft = left[:, 0, :, 111:161]    # [B, W, 50]
    rgt = right[:, 0, :, 111:161]
    refs = np.empty((NCORES, KPC, 4, 128, 50), np.float32)
    for core in range(NCORES):
        for k in range(KPC):
            kg = core * KPC + k
            refs[core, k, 0] = lft[kg, 0:128, :]
            refs[core, k, 1] = rgt[kg, 0:128, :]
            refs[core, k, 2] = lft[kg, 128:256, :]
            refs[core, k, 3] = rgt[kg, 128:256, :]
    refs = refs * 60.0 + np.float32(OFF)
    # [core, chan, k, blk, bin]
    refs = refs.transpose(0, 3, 1, 2, 4)
    return np.ascontiguousarray(
        refs.reshape(NCORES, 128, TBL_W).astype(np.float16)
    )


def make_in_maps(up, left, right):
    up = np.asarray(up, np.float32)
    left = np.asarray(left, np.float32)
    right = np.asarray(right, np.float32)
    vee, ident, antid = _host_constants()
    negid = np.ascontiguousarray(-ident)
    refs = _prep_refs(left, right)
    bcol = np.ascontiguousarray(np.broadcast_to(
        np.array([12.0, -11.99609375, 0.0, -0.0235], np.float32), (128, 4)))
    in_maps = []
    for c in range(NCORES):
        upc = np.ascontiguousarray(
            up[c * KPC:(c + 1) * KPC, 0].reshape(KPC * H, W)
        )
        in_maps.append({
            "up_in": upc,
            "bcol_in": bcol,
            "refs_in": refs[c],
            "vee_in": vee,
            "ident_in": ident,
            "antid_in": antid,
            "negid_in": negid,
        })
    return in_maps


def get_nc():
    if "nc" not in _cache:
        _cache["nc"] = _build_bass()
    return _cache["nc"]


def reduce_results(results):
    # per chunk c: parts[:, 2c] = R = sum(relu(12 - a)),
    #              parts[:, 2c+1] = N = sum(a < 11.996...)
    # S = 12N - R
    total = 0.0
    for r in results:
        o = np.asarray(r["out"]).astype(np.float64)
        for c in range(NCHUNK):
            rsum = o[:, 2 * c].sum()
            ncnt = o[:, 2 * c + 1].sum()
            total += 12.0 * ncnt - rsum
    return np.float32(total / (60.0 * B * W * W))


def kernel(up, left, right):
    nc = get_nc()
    in_maps = make_in_maps(up, left, right)
    res = run_bass_kernel_spmd(nc, in_maps, core_ids=list(range(NCORES)))
    return reduce_results(res.results)
